# revision 7
# baseline (speedup 1.0000x reference)
"""Trainium2 Bass kernel for nn_CompetitiveLayer (fixed-point competitive layer).

Algorithm (reference):
    K = param**2
    repeat 21x:  AF = AT / (1 + K @ BF);  BF = BT / (1 + AF @ K)
    C = K * AF[:, None] * BF[None, :]

Distribution: K is sharded row-wise over 8 cores (512 rows each). Each core
keeps its K-slice SBUF-resident in two layouts (8 MB each):
  kt_sb[p, c, n] = K[512*i + n, 32*p + c]   (for u = K_i @ BF, contraction on nB)
  k_sb [p, m, k] = K[512*i + 128*m + p, k]  (for partial = K_i^T @ AF_i)
Matvecs run on the PE with the vector as the stationary operand (M=1) and the
matrix slice as the moving operand (N=512) - the stream-port-bound orientation.
The BF update needs a 16 KB AllReduce of partial K^T AF sums per iteration.
"""

import numpy as np
import os
import sys

for _p in ("/opt/trn_rl_repo",):
    if _p not in sys.path and os.path.isdir(_p):
        sys.path.insert(0, _p)

N = 4096          # nA == nB
NCORES = 8
R = N // NCORES   # 512 rows per core
ITERS = 21        # 20 scan iterations + 1 last_iterate pass

_NC_CACHE = {}
LAST_RESULTS = None


def build_nc(iters=ITERS, n=N, ncores=NCORES):
    import concourse.bass as bass
    import concourse.mybir as mybir
    import concourse.tile as tile

    f32 = mybir.dt.float32
    r = n // ncores          # local rows
    M4 = r // 128            # row chunks of 128 (4)
    C32 = n // 128           # contraction chunks of 128 over nB (32)
    B8 = n // 512            # 512-wide column blocks of nB (8)
    groups = [list(range(ncores))]

    nc = bass.Bass(num_devices=ncores)

    kp = nc.dram_tensor("kp", [128, M4, n], f32, kind="ExternalInput")
    ktp = nc.dram_tensor("ktp", [128, C32, r], f32, kind="ExternalInput")
    att = nc.dram_tensor("att", [128, M4], f32, kind="ExternalInput")
    atf = nc.dram_tensor("atf", [1, r], f32, kind="ExternalInput")
    btt = nc.dram_tensor("btt", [128, n // 128], f32, kind="ExternalInput")
    c_out = nc.dram_tensor("c_out", [r, n], f32, kind="ExternalOutput")

    with tile.TileContext(nc) as tc:
        with (
            tc.tile_pool(name="kbig", bufs=1) as kbig,
            tc.tile_pool(name="vecs", bufs=1) as vecs,
            tc.tile_pool(name="small", bufs=3) as small,
            tc.tile_pool(name="csb", bufs=4) as csb,
            tc.tile_pool(name="psu", bufs=2, space="PSUM") as psu,
            tc.tile_pool(name="pst", bufs=2, space="PSUM") as pst,
            tc.tile_pool(name="psp", bufs=3, space="PSUM") as psp,
            tc.tile_pool(name="dram", bufs=3, space="DRAM") as dram,
        ):
            k_sb = kbig.tile([128, M4, n], f32)
            kt_sb = kbig.tile([128, C32, r], f32)
            att_sb = vecs.tile([128, M4], f32)
            atf_sb = vecs.tile([1, r], f32)
            btt_sb = vecs.tile([128, n // 128], f32)
            one_sb = vecs.tile([1, 1], f32)

            nc.sync.dma_start(att_sb[:], att[:])
            nc.sync.dma_start(atf_sb[:], atf[:])
            nc.sync.dma_start(btt_sb[:], btt[:])
            nc.vector.memset(one_sb[:], 1.0)

            # Load K slices and square in place, chunked so squaring (split
            # between ACT and DVE) overlaps the DMAs.
            sq = 0
            for m in range(M4):
                for h in range(2):
                    sl = (slice(None), m, slice(h * (n // 2), (h + 1) * (n // 2)))
                    nc.sync.dma_start(k_sb[sl], kp[sl])
                    if sq % 2 == 0:
                        nc.scalar.square(k_sb[sl], k_sb[sl])
                    else:
                        nc.vector.tensor_mul(k_sb[sl], k_sb[sl], k_sb[sl])
                    sq += 1
            for g in range(8):
                cs = C32 // 8
                sl = (slice(None), slice(g * cs, (g + 1) * cs), slice(None))
                nc.sync.dma_start(kt_sb[sl], ktp[sl])
                if sq % 2 == 0:
                    nc.scalar.square(kt_sb[sl], kt_sb[sl])
                else:
                    nc.vector.tensor_mul(kt_sb[sl], kt_sb[sl], kt_sb[sl])
                sq += 1

            bf = btt_sb  # BF_0 = BT
            u_sb = None
            for t in range(iters):
                # ---- u = K_i @ BF  -> [1, r] on partition 0 ----
                u_ps = psu.tile([1, r], f32, tag="u", name=f"u_ps_{t}")
                for c in range(C32):
                    nc.tensor.matmul(
                        u_ps[:],
                        bf[:, c : c + 1],
                        kt_sb[:, c, :],
                        start=(c == 0),
                        stop=(c == C32 - 1),
                    )
                u_sb = small.tile([1, r], f32, tag="usb", bufs=2, name=f"u_sb_{t}")
                nc.scalar.copy(u_sb[:], u_ps[:])

                # ---- transpose u to partitions: uT[p, m] = u[128m+p] ----
                uT_ps = pst.tile([128, M4], f32, tag="uT", name=f"uT_ps_{t}")
                for m in range(M4):
                    nc.tensor.matmul(
                        uT_ps[:, m : m + 1],
                        u_sb[0:1, 128 * m : 128 * (m + 1)],
                        one_sb[:],
                    )

                # ---- AF = AT / (1 + u) in [128, M4] chunk-major layout ----
                af_sb = small.tile([128, M4], f32, tag="af", name=f"af_sb_{t}")
                nc.vector.tensor_scalar_add(af_sb[:], uT_ps[:], 1.0)
                nc.vector.reciprocal(af_sb[:], af_sb[:])
                nc.vector.tensor_mul(af_sb[:], af_sb[:], att_sb[:])

                # ---- partial = K_i^T @ AF_i -> [1, n], DMA'd per 512-block ----
                cc_in = dram.tile([1, n], f32, tag="ccin", name=f"cc_in_{t}")
                cc_out = dram.tile(
                    [1, n], f32, tag="ccout", addr_space="Shared", name=f"cc_out_{t}"
                )
                p_sb = small.tile([1, n], f32, tag="psb", bufs=1, name=f"p_sb_{t}")
                for b in range(B8):
                    pb_ps = psp.tile([1, 512], f32, tag="pblk", name=f"pb_ps_{t}_{b}")
                    for m in range(M4):
                        nc.tensor.matmul(
                            pb_ps[:],
                            af_sb[:, m : m + 1],
                            k_sb[:, m, 512 * b : 512 * (b + 1)],
                            start=(m == 0),
                            stop=(m == M4 - 1),
                        )
                    nc.scalar.copy(p_sb[0:1, 512 * b : 512 * (b + 1)], pb_ps[:])
                    nc.sync.dma_start(
                        cc_in[0:1, 512 * b : 512 * (b + 1)],
                        p_sb[0:1, 512 * b : 512 * (b + 1)],
                    )

                # ---- AllReduce partials across cores ----
                nc.gpsimd.collective_compute(
                    "AllReduce",
                    mybir.AluOpType.add,
                    replica_groups=groups,
                    ins=[cc_in[:]],
                    outs=[cc_out[:]],
                )

                # ---- BF = BT / (1 + s) in [128, n/128] layout: bf[p,c]=BF[32p+c]
                s_sb = small.tile([128, n // 128], f32, tag="ssb", name=f"s_sb_{t}")
                nc.sync.dma_start(
                    s_sb[:], cc_out[0, :].rearrange("(p c) -> p c", p=128)
                )
                bf2 = small.tile([128, n // 128], f32, tag="bf", name=f"bf_sb_{t}")
                nc.vector.tensor_scalar_add(bf2[:], s_sb[:], 1.0)
                nc.vector.reciprocal(bf2[:], bf2[:])
                nc.vector.tensor_mul(bf2[:], bf2[:], btt_sb[:])
                bf = bf2

            # ---- finale: C = K * AF ⊗ BF ----
            # AF in natural free layout from the last u
            af_free = vecs.tile([1, r], f32)
            nc.vector.tensor_scalar_add(af_free[:], u_sb[:], 1.0)
            nc.vector.reciprocal(af_free[:], af_free[:])
            nc.vector.tensor_mul(af_free[:], af_free[:], atf_sb[:])
            # BF natural free layout via a DRAM round-trip (layout shuffle)
            bfx = dram.tile([1, n], f32, tag="bfx")
            nc.sync.dma_start(bfx[0, :].rearrange("(p c) -> p c", p=128), bf[:])
            bf_free = vecs.tile([1, n], f32)
            nc.sync.dma_start(bf_free[:], bfx[:])

            for m in range(M4):
                for b in range(B8):
                    o_ps = psp.tile([128, 512], f32, tag="pblk", name=f"o_ps_{m}_{b}")
                    nc.tensor.matmul(
                        o_ps[:],
                        af_free[0:1, 128 * m : 128 * (m + 1)],
                        bf_free[0:1, 512 * b : 512 * (b + 1)],
                    )
                    c_sb = csb.tile([128, 512], f32, tag="c", name=f"c_sb_{m}_{b}")
                    nc.vector.tensor_mul(
                        c_sb[:], k_sb[:, m, 512 * b : 512 * (b + 1)], o_ps[:]
                    )
                    nc.sync.dma_start(
                        c_out[128 * m : 128 * (m + 1), 512 * b : 512 * (b + 1)],
                        c_sb[:],
                    )

    return nc


def _legalize_multiwait(nc):
    """This walrus build accepts at most ONE sync wait per instruction.
    Split multi-wait instructions: keep one wait, hoist the rest onto
    single-wait NoOps inserted immediately before on the same engine
    (engines are in-order, so this is equivalent)."""
    import concourse.mybir as mybir

    uid = [0]
    for fn in nc.m.functions:
        for blk in fn.blocks:
            insts = list(blk.instructions)
            out = []
            changed = False
            for ins in insts:
                si = ins.sync_info
                if si is not None and si.on_wait and len(si.on_wait) > 1:
                    waits = list(si.on_wait)
                    for w in waits[:-1]:
                        uid[0] += 1
                        nop = mybir.InstNoOp(
                            name=f"I-mwfix-{uid[0]}", ins=[], outs=[]
                        )
                        nop.engine = ins.engine
                        nop.sync_info = mybir.SyncInfo(on_wait=[w], on_update=[])
                        out.append(nop)
                    ins.sync_info = mybir.SyncInfo(
                        on_wait=[waits[-1]], on_update=list(si.on_update or [])
                    )
                    changed = True
                out.append(ins)
            if changed:
                try:
                    blk.instructions = out
                except Exception:
                    blk.instructions.clear()
                    blk.instructions.extend(out)


def make_in_maps(AT, BT, param, n=N, ncores=NCORES):
    AT = np.ascontiguousarray(AT, dtype=np.float32)
    BT = np.ascontiguousarray(BT, dtype=np.float32)
    param = np.ascontiguousarray(param, dtype=np.float32)
    r = n // ncores
    btt = np.ascontiguousarray(BT.reshape(128, n // 128))
    in_maps = []
    for i in range(ncores):
        rows = param[i * r : (i + 1) * r, :]                      # [r, n]
        kp = np.ascontiguousarray(
            rows.reshape(r // 128, 128, n).transpose(1, 0, 2)
        )                                                         # [128, r/128, n]
        ktp = np.ascontiguousarray(rows.T).reshape(128, n // 128, r)
        att = np.ascontiguousarray(
            AT[i * r : (i + 1) * r].reshape(r // 128, 128).T
        )                                                         # [128, r/128]
        atf = np.ascontiguousarray(AT[i * r : (i + 1) * r].reshape(1, r))
        in_maps.append({"kp": kp, "ktp": ktp, "att": att, "atf": atf, "btt": btt})
    return in_maps


def kernel(AT, BT, param):
    global LAST_RESULTS
    from concourse.bass_utils import run_bass_kernel_spmd

    AT = np.asarray(AT, dtype=np.float32)
    BT = np.asarray(BT, dtype=np.float32)
    param = np.asarray(param, dtype=np.float32)

    key = (ITERS, N, NCORES)
    if key not in _NC_CACHE:
        nc = build_nc(*key)
        _legalize_multiwait(nc)
        _NC_CACHE[key] = nc
    nc = _NC_CACHE[key]

    in_maps = make_in_maps(AT, BT, param)
    res = run_bass_kernel_spmd(nc, in_maps, core_ids=list(range(NCORES)))
    LAST_RESULTS = res
    C = np.concatenate([res.results[i]["c_out"] for i in range(NCORES)], axis=0)
    return np.ascontiguousarray(C, dtype=np.float32)


if __name__ == "__main__":
    rng = np.random.RandomState(0)
    AT = rng.uniform(0, 1, N).astype(np.float32)
    BT = rng.uniform(0, 1, N).astype(np.float32)
    param = rng.uniform(0, 1, (N, N)).astype(np.float32)
    C = kernel(AT, BT, param)
    K = param * param
    AF, BF = AT.copy(), BT.copy()
    for _ in range(ITERS):
        AF = AT / (1.0 + K @ BF)
        BF = BT / (1.0 + AF @ K)
    ref = K * AF[:, None] * BF[None, :]
    err = np.abs(C - ref).max() / np.abs(ref).max()
    print("scale-relative absmax err:", err)


# revision 30
# speedup vs baseline: 1.9389x; 1.9389x over previous
"""Trainium2 Bass kernel for nn_CompetitiveLayer (fixed-point competitive layer).

Algorithm (reference):
    K = param**2
    repeat 21x:  AF = AT / (1 + K @ BF);  BF = BT / (1 + AF @ K)
    C = K * AF[:, None] * BF[None, :]

Distribution: K is sharded row-wise over 8 cores (512 rows each). Each core
keeps its K-slice SBUF-resident in two layouts (8 MB each):
  kt_sb[p, c, n] = K[512*i + n, 32*p + c]   (for u = K_i @ BF, contraction on nB)
  k_sb [p, m, k] = K[512*i + 128*m + p, k]  (for partial = K_i^T @ AF_i)
Matvecs run on the PE with the vector as the stationary operand (M=1) and the
matrix slice as the moving operand (N=512) - the stream-port-bound orientation.
The BF update needs a 16 KB AllReduce of partial K^T AF sums per iteration.
"""

import numpy as np
import os
import sys

for _p in ("/opt/trn_rl_repo",):
    if _p not in sys.path and os.path.isdir(_p):
        sys.path.insert(0, _p)

N = 4096          # nA == nB
NCORES = 8
R = N // NCORES   # 512 rows per core
ITERS = 21        # 20 scan iterations + 1 last_iterate pass

_NC_CACHE = {}
LAST_RESULTS = None


def build_nc(iters=ITERS, n=N, ncores=NCORES, no_cc=False):
    import concourse.bass as bass
    import concourse.mybir as mybir
    import concourse.tile as tile

    f32 = mybir.dt.float32
    bf16 = mybir.dt.bfloat16
    r = n // ncores          # local rows
    M4 = r // 128            # row chunks of 128 (4)
    C32 = n // 128           # contraction chunks of 128 over nB (32)
    B8 = n // 512            # 512-wide column blocks of nB (8)
    groups = [list(range(ncores))]

    nc = bass.Bass(num_devices=ncores)

    kp = nc.dram_tensor("kp", [128, M4, n], f32, kind="ExternalInput")
    ktp = nc.dram_tensor("ktp", [128, C32, r], f32, kind="ExternalInput")
    att = nc.dram_tensor("att", [128, M4], f32, kind="ExternalInput")
    atf = nc.dram_tensor("atf", [1, r], f32, kind="ExternalInput")
    btt = nc.dram_tensor("btt", [128, n // 128], f32, kind="ExternalInput")
    c_out = nc.dram_tensor("c_out", [r, n], f32, kind="ExternalOutput")

    with tile.TileContext(nc) as tc:
        with (
            tc.tile_pool(name="kbig", bufs=1) as kbig,
            tc.tile_pool(name="vecs", bufs=1) as vecs,
            tc.tile_pool(name="small", bufs=3) as small,
            tc.tile_pool(name="csb", bufs=4) as csb,
            tc.tile_pool(name="psu", bufs=2, space="PSUM") as psu,
            tc.tile_pool(name="pst", bufs=2, space="PSUM") as pst,
            tc.tile_pool(name="psp", bufs=3, space="PSUM") as psp,
            tc.tile_pool(name="dram", bufs=3, space="DRAM") as dram,
        ):
            k_sb = kbig.tile([128, M4, n], f32)      # fp32 K rows (final C)
            k16 = kbig.tile([128, M4, n], bf16)      # bf16 K rows (mv_B)
            kt16 = kbig.tile([128, C32, r], bf16)    # bf16 K^T (mv_A)
            att_sb = vecs.tile([128, M4], f32)
            atf_sb = vecs.tile([1, r], f32)
            btt_sb = vecs.tile([128, n // 128], f32)
            btt16 = vecs.tile([128, n // 128], bf16)
            one_sb = vecs.tile([1, 1], f32)

            nc.sync.dma_start(att_sb[:], att[:])
            nc.sync.dma_start(atf_sb[:], atf[:])
            nc.sync.dma_start(btt_sb[:], btt[:])
            nc.vector.tensor_copy(btt16[:], btt_sb[:])
            nc.vector.memset(one_sb[:], 1.0)

            # Load K slices chunked. K^T (bf16, gates the first matvec) goes
            # first through rotating fp32 temps with a fused square+cast,
            # alternating ACT/DVE. Then K rows: square fp32 in place (ACT)
            # and cast a bf16 copy (DVE).
            for g in range(8):
                cs = C32 // 8
                sl = (slice(None), slice(g * cs, (g + 1) * cs), slice(None))
                tkt = small.tile([128, cs, r], f32, tag="tmpkt", name=f"tkt_{g}")
                nc.sync.dma_start(tkt[:], ktp[sl])
                if g % 2 == 0:
                    nc.scalar.square(kt16[sl], tkt[:])
                else:
                    nc.vector.tensor_mul(kt16[sl], tkt[:], tkt[:])
            for m in range(M4):
                for h in range(2):
                    sl = (slice(None), m, slice(h * (n // 2), (h + 1) * (n // 2)))
                    nc.sync.dma_start(k_sb[sl], kp[sl])
                    nc.scalar.square(k_sb[sl], k_sb[sl])
                    nc.vector.tensor_copy(k16[sl], k_sb[sl])
            bf = btt16  # BF_0 = BT
            u_sb = None
            for t in range(iters):
                # ---- u = K_i @ BF  -> [1, r] on partition 0 ----
                u_ps = psu.tile([1, r], f32, tag="u", name=f"u_ps_{t}")
                for c in range(C32):
                    nc.tensor.matmul(
                        u_ps[:],
                        bf[:, c : c + 1],
                        kt16[:, c, :],
                        start=(c == 0),
                        stop=(c == C32 - 1),
                    )
                u_sb = small.tile([1, r], f32, tag="usb", bufs=2, name=f"u_sb_{t}")
                nc.scalar.copy(u_sb[:], u_ps[:])

                # ---- transpose u to partitions: uT[p, m] = u[128m+p] ----
                uT_ps = pst.tile([128, M4], f32, tag="uT", name=f"uT_ps_{t}")
                for m in range(M4):
                    nc.tensor.matmul(
                        uT_ps[:, m : m + 1],
                        u_sb[0:1, 128 * m : 128 * (m + 1)],
                        one_sb[:],
                    )

                # ---- AF = AT / (1 + u) in [128, M4] chunk-major layout ----
                afr = small.tile([128, M4], f32, tag="af", name=f"afr_{t}")
                nc.vector.tensor_scalar_add(afr[:], uT_ps[:], 1.0)
                nc.vector.reciprocal(afr[:], afr[:])
                af16 = small.tile([128, M4], bf16, tag="af16", name=f"af16_{t}")
                nc.vector.tensor_mul(af16[:], afr[:], att_sb[:])

                # ---- partial = K_i^T @ AF_i -> [1, n], AllReduduced in 4
                # column-quarters so each AR overlaps remaining PE work and
                # the next iteration's mv_A starts as quarters land. ----
                p_sb = small.tile([1, n], f32, tag="psb", bufs=1, name=f"p_sb_{t}")
                s_sb = small.tile([128, n // 128], f32, tag="ssb", name=f"s_sb_{t}")
                if t == iters - 1:
                    bf2 = small.tile(
                        [128, n // 128], f32, tag="bf", bufs=1, name=f"bf_sb_{t}"
                    )
                bf16t = small.tile([128, n // 128], bf16, tag="bf16", name=f"bf16_{t}")
                nq = n // 4  # 1024 elements per AR quarter
                cq = nq // 128  # 8 contraction chunks per quarter
                for q in range(4):
                    for b in (2 * q, 2 * q + 1):
                        pb_ps = psp.tile(
                            [1, 512], f32, tag="pblk", name=f"pb_ps_{t}_{b}"
                        )
                        for m in range(M4):
                            nc.tensor.matmul(
                                pb_ps[:],
                                af16[:, m : m + 1],
                                k16[:, m, 512 * b : 512 * (b + 1)],
                                start=(m == 0),
                                stop=(m == M4 - 1),
                            )
                        nc.scalar.copy(p_sb[0:1, 512 * b : 512 * (b + 1)], pb_ps[:])
                    cc_in = dram.tile(
                        [1, nq], f32, tag=f"ccin{q}", name=f"cc_in_{t}_{q}"
                    )
                    cc_out = dram.tile(
                        [1, nq], f32, tag=f"ccout{q}", addr_space="Shared",
                        name=f"cc_out_{t}_{q}",
                    )
                    nc.sync.dma_start(cc_in[:], p_sb[0:1, nq * q : nq * (q + 1)])
                    if no_cc:
                        nc.sync.dma_start(cc_out[:], cc_in[:])
                    else:
                        nc.gpsimd.collective_compute(
                            "AllReduce",
                            mybir.AluOpType.add,
                            replica_groups=groups,
                            ins=[cc_in[:]],
                            outs=[cc_out[:]],
                        )
                    # BF quarter: bf[p, c] = BT[128c+p] / (1 + s[128c+p])
                    qs = slice(cq * q, cq * (q + 1))
                    nc.sync.dma_start(
                        s_sb[:, qs], cc_out[0, :].rearrange("(c p) -> p c", p=128)
                    )
                    nc.vector.tensor_scalar_add(s_sb[:, qs], s_sb[:, qs], 1.0)
                    nc.vector.reciprocal(s_sb[:, qs], s_sb[:, qs])
                    nc.vector.tensor_mul(bf16t[:, qs], s_sb[:, qs], btt_sb[:, qs])
                    if t == iters - 1:
                        nc.vector.tensor_mul(
                            bf2[:, qs], s_sb[:, qs], btt_sb[:, qs]
                        )
                # Keep the PE busy during the AllReduce flight so HAM stays
                # at full clock (an idle window >3.4us halves the PE clock
                # for the next ~3.4us). Harmless fp32 copies of p_sb through
                # the PE, gated on mv_B's output so they fill the gap.
                if t < iters - 1:
                    warm_ps = psu.tile([1, 512], f32, tag="u", name=f"warm_{t}")
                    for w in range(10):
                        nc.tensor.matmul(
                            warm_ps[0:1, 0:256],
                            one_sb[:],
                            p_sb[0:1, 256 * (w % 8) : 256 * (w % 8) + 256],
                        )
                bf = bf16t
                if t == iters - 1:
                    bf_f32 = bf2

            # ---- finale: C = K * AF ⊗ BF ----
            # AF in natural free layout from the last u
            af_free = vecs.tile([1, r], f32)
            nc.vector.tensor_scalar_add(af_free[:], u_sb[:], 1.0)
            nc.vector.reciprocal(af_free[:], af_free[:])
            nc.vector.tensor_mul(af_free[:], af_free[:], atf_sb[:])
            # BF natural free layout via a DRAM round-trip (layout shuffle)
            bfx = dram.tile([1, n], f32, tag="bfx")
            nc.sync.dma_start(bfx[0, :].rearrange("(c p) -> p c", p=128), bf_f32[:])
            bf_free = vecs.tile([1, n], f32)
            nc.sync.dma_start(bf_free[:], bfx[:])

            for m in range(M4):
                for b in range(B8):
                    o_ps = psp.tile([128, 512], f32, tag="pblk", name=f"o_ps_{m}_{b}")
                    nc.tensor.matmul(
                        o_ps[:],
                        af_free[0:1, 128 * m : 128 * (m + 1)],
                        bf_free[0:1, 512 * b : 512 * (b + 1)],
                    )
                    c_sb = csb.tile([128, 512], f32, tag="c", name=f"c_sb_{m}_{b}")
                    nc.vector.tensor_mul(
                        c_sb[:], k_sb[:, m, 512 * b : 512 * (b + 1)], o_ps[:]
                    )
                    nc.sync.dma_start(
                        c_out[128 * m : 128 * (m + 1), 512 * b : 512 * (b + 1)],
                        c_sb[:],
                    )

    return nc


def _legalize_multiwait(nc):
    """This walrus build accepts at most ONE sync wait per instruction.
    Split multi-wait instructions: keep one wait, hoist the rest onto
    single-wait NoOps inserted immediately before on the same engine
    (engines are in-order, so this is equivalent)."""
    import concourse.mybir as mybir

    uid = [0]
    for fn in nc.m.functions:
        for blk in fn.blocks:
            insts = list(blk.instructions)
            out = []
            changed = False
            for ins in insts:
                si = ins.sync_info
                if si is not None and si.on_wait and len(si.on_wait) > 1:
                    waits = list(si.on_wait)
                    for w in waits[:-1]:
                        uid[0] += 1
                        nop = mybir.InstNoOp(
                            name=f"I-mwfix-{uid[0]}", ins=[], outs=[]
                        )
                        nop.engine = ins.engine
                        nop.sync_info = mybir.SyncInfo(on_wait=[w], on_update=[])
                        out.append(nop)
                    ins.sync_info = mybir.SyncInfo(
                        on_wait=[waits[-1]], on_update=list(si.on_update or [])
                    )
                    changed = True
                out.append(ins)
            if changed:
                try:
                    blk.instructions = out
                except Exception:
                    blk.instructions.clear()
                    blk.instructions.extend(out)


def make_in_maps(AT, BT, param, n=N, ncores=NCORES):
    AT = np.ascontiguousarray(AT, dtype=np.float32)
    BT = np.ascontiguousarray(BT, dtype=np.float32)
    param = np.ascontiguousarray(param, dtype=np.float32)
    r = n // ncores
    btt = np.ascontiguousarray(BT.reshape(n // 128, 128).T)
    in_maps = []
    for i in range(ncores):
        rows = param[i * r : (i + 1) * r, :]                      # [r, n]
        kp = np.ascontiguousarray(
            rows.reshape(r // 128, 128, n).transpose(1, 0, 2)
        )                                                         # [128, r/128, n]
        ktp = np.ascontiguousarray(
            np.ascontiguousarray(rows.T)
            .reshape(n // 128, 128, r)
            .transpose(1, 0, 2)
        )                                                         # [128, n/128, r]
        att = np.ascontiguousarray(
            AT[i * r : (i + 1) * r].reshape(r // 128, 128).T
        )                                                         # [128, r/128]
        atf = np.ascontiguousarray(AT[i * r : (i + 1) * r].reshape(1, r))
        in_maps.append({"kp": kp, "ktp": ktp, "att": att, "atf": atf, "btt": btt})
    return in_maps


def kernel(AT, BT, param):
    global LAST_RESULTS
    from concourse.bass_utils import run_bass_kernel_spmd

    AT = np.asarray(AT, dtype=np.float32)
    BT = np.asarray(BT, dtype=np.float32)
    param = np.asarray(param, dtype=np.float32)

    key = (ITERS, N, NCORES)
    if key not in _NC_CACHE:
        nc = build_nc(*key)
        _legalize_multiwait(nc)
        _NC_CACHE[key] = nc
    nc = _NC_CACHE[key]

    in_maps = make_in_maps(AT, BT, param)
    res = run_bass_kernel_spmd(nc, in_maps, core_ids=list(range(NCORES)))
    LAST_RESULTS = res
    C = np.concatenate([res.results[i]["c_out"] for i in range(NCORES)], axis=0)
    return np.ascontiguousarray(C, dtype=np.float32)


if __name__ == "__main__":
    rng = np.random.RandomState(0)
    AT = rng.uniform(0, 1, N).astype(np.float32)
    BT = rng.uniform(0, 1, N).astype(np.float32)
    param = rng.uniform(0, 1, (N, N)).astype(np.float32)
    C = kernel(AT, BT, param)
    K = param * param
    AF, BF = AT.copy(), BT.copy()
    for _ in range(ITERS):
        AF = AT / (1.0 + K @ BF)
        BF = BT / (1.0 + AF @ K)
    ref = K * AF[:, None] * BF[None, :]
    err = np.abs(C - ref).max() / np.abs(ref).max()
    print("scale-relative absmax err:", err)
